# revision 14
# baseline (speedup 1.0000x reference)
"""ChannelCovarianceBlock Trainium2 kernel (fp8 DoubleRow version).

Computes, for queries x1 (B, C, h, w) and support sets x2 (nw, Bs, C, h, w):
  cov_n = Cov(x2[n].reshape(Bs*C, hw))            (hw, hw) per class
  d     = normalize-and-center rows of x1.reshape(B*C, hw)
  sim[b, n, c] = d[bc] @ cov_n @ d[bc]^T          -> (B, nw*C)

Sharding: data-parallel over B across 8 NeuronCores (32 queries each);
each core computes all 10 class covariances from the full x2 (redundant
but collective-free) using the Gram identity cov = (X^T X - s s^T/N)/(N-1).

Numerics: matmuls run in fp8e4 (e4m3) with MatmulPerfMode.DoubleRow
(0.5 PE cycles/row, 2x bf16 throughput). To survive fp8's 3-bit
mantissa, the covariance is split as cov = I + V: the exact base term
||d||^2 = 1 - hw*m^2 (m = row mean of the normalized query) is computed
from stage-0 stats in f32, and only the small-valued V = cov - I is
quantized to fp8 (the I subtraction happens inside PSUM via an exact
f32r matmul against a shifted-identity tile). d is scaled by 16 and V
by 16 before fp8 quantization; the 1/256 descale folds into the final
multiply-reduce. Measured rel err ~8e-3 (numpy model 7.8e-3).

Per-core dataflow (single pass, no DRAM spills):
  phase A (interleaved per m-tile / per class to keep PE busy):
    stage 0: load x1 rows (bf16), compute norm/mean stats, write the
             normalized d as bf16 (d_res, stt operand) and build D^T in
             fp8 via PE transposes (dtT, matmul lhsT), both SBUF-resident.
    gram:    per class, 5 DoubleRow Gram matmuls per (k-block, q-chunk)
             + rank-1 mean correction + f32r -(NR-1)*I matmul, then
             copy PSUM -> V fp8 (all 10 classes stay resident).
  phase B: per (class, m-tile): 8 matmuls (6 DoubleRow + 2 k=16
           remainder) -> S = D @ V in PSUM, then one DVE
           scalar_tensor_tensor (S * d_res, accum_out) -> out_acc.
           Finally out_acc += base, DMA out.
"""

import os
import sys

for _p in ("/opt/trn_rl_repo", "/root/.axon_site/_ro/trn_rl_repo"):
    if os.path.isdir(_p) and _p not in sys.path:
        sys.path.append(_p)

import numpy as np

# Problem constants (hardcoded per spec).
B, C, H, W = 256, 128, 28, 28
NW, BS = 10, 10
HW = H * W            # 784
N_CORES = 8
BSH = B // N_CORES    # 32 queries per core
NI = BSH * C          # 4096 rows per core
NR = BS * C           # 1280 support rows per class
RTN = NR // 128       # 10 row-tiles per class

# K-tiles over the hw contraction dim (partition dim <= 128).
KT = [(k * 128, min(128, HW - k * 128)) for k in range((HW + 127) // 128)]
NKT = len(KT)         # 7 (6 full + 16-row remainder)
NKT8 = 8              # k-tile slots incl. zero pad so kt (6,7) forms a DR pair
NDR = 3               # DoubleRow k-tile pairs (0,1)(2,3)(4,5); kt=6 plain
QT = [(0, 392), (392, 392)]
MT = NI // 128        # 32 i-tiles per core

SD = 16.0             # d scale before fp8 quantization
SC = 16.0             # V scale before fp8 quantization
EYE_OFF = 384         # identity block column offset in the EYE tile

_STATE = {}


def _build_program(repeat=None, abl=None):
    if repeat is None:
        repeat = int(os.environ.get("CCB_REPEAT", "1"))
    if abl is None:
        abl = os.environ.get("CCB_ABL", "full")
    import concourse.bass as bass
    import concourse.bacc as bacc
    import concourse.tile as tile
    from concourse import mybir
    from concourse.masks import make_identity
    from contextlib import ExitStack

    f32 = mybir.dt.float32
    f32r = mybir.dt.float32r
    bf16 = mybir.dt.bfloat16
    fp8 = mybir.dt.float8e4
    DRM = mybir.MatmulPerfMode.DoubleRow
    ALPHA = float(np.sqrt(NR - 1.0))

    nc = bacc.Bacc()
    x1s = nc.declare_dram_parameter("x1s", [NI, HW], bf16, isOutput=False)
    x2d = nc.declare_dram_parameter("x2", [NW, RTN, 128, HW], fp8, isOutput=False)
    out = nc.declare_dram_parameter("out", [MT, 128, NW], f32, isOutput=True)
    debug = os.environ.get("CCB_DEBUG") == "1"
    if debug:
        dbg_dtT = nc.declare_dram_parameter(
            "dbg_dtT", [128, MT, NKT8, 128], fp8, isOutput=True)
        dbg_cov = nc.declare_dram_parameter(
            "dbg_cov", [128, NW, NKT8, HW], fp8, isOutput=True)
        dbg_base = nc.declare_dram_parameter(
            "dbg_base", [128, MT], f32, isOutput=True)
        dbg_dres = nc.declare_dram_parameter(
            "dbg_dres", [128, MT, 2, 392], bf16, isOutput=True)

    AF = mybir.ActivationFunctionType
    OP = mybir.AluOpType

    with tile.TileContext(nc) as tc:
        with ExitStack() as ctx:
            persist = ctx.enter_context(tc.tile_pool(name="persist", bufs=1))
            ident_f = persist.tile([128, 128], f32, tag="ident_f")
            make_identity(nc, ident_f)
            # f32r copies must come from walrus-approved producers (DVE).
            ident = persist.tile([128, 128], f32r, tag="ident")
            nc.vector.tensor_copy(out=ident, in_=ident_f)
            # AI = +alpha*I, EYE carries -alpha at [p, EYE_OFF+p]; their
            # product in PSUM subtracts (NR-1)*I from the Gram exactly.
            ai = persist.tile([128, 128], f32r, tag="ai")
            nc.vector.tensor_scalar(
                out=ai, in0=ident_f, scalar1=ALPHA, scalar2=None, op0=OP.mult
            )
            eye_f = persist.tile([128, HW], f32, tag="eye_f")
            nc.vector.memset(eye_f, 0.0)
            nc.vector.tensor_scalar(
                out=eye_f[:, EYE_OFF:EYE_OFF + 128], in0=ident_f,
                scalar1=-ALPHA, scalar2=None, op0=OP.mult,
            )
            eye = persist.tile([128, HW], f32r, tag="eye")
            nc.vector.tensor_copy(out=eye, in_=eye_f)
            # DR weight APs need even, 16B-aligned outer free steps.
            ones2 = persist.tile([128, 2, 16], fp8, tag="ones2")
            nc.vector.memset(ones2, 1.0)
            # stt operand: normalized d, bf16, (2, 392) layout to match psum
            d_res = persist.tile([128, MT, 2, 392], bf16, tag="d_res")
            # matmul lhsT: D^T in fp8, scaled by SD
            dtT = persist.tile([128, MT, NKT8, 128], fp8, tag="dtT")
            # all 10 class V matrices, fp8, scaled by SC
            cov = persist.tile([128, NW, NKT8, HW], fp8, tag="cov")
            # zero the DR pad: kt=7 plane and partitions 16.. of kt=6
            nc.vector.memset(dtT[:, :, NKT8 - 1, :], 0.0)
            nc.vector.memset(dtT[:, :, NKT - 1, :], 0.0)
            nc.vector.memset(cov[:, :, NKT8 - 1, :], 0.0)
            nc.vector.memset(cov[:, :, NKT - 1, :], 0.0)
            out_acc = persist.tile([128, MT, NW], f32, tag="out_acc")
            base_t = persist.tile([128, MT], f32, tag="base")

            sumsq_all = persist.tile([128, MT], f32, tag="sumsq")
            s1_all = persist.tile([128, MT], f32, tag="s1")
            nrm_all = persist.tile([128, MT], f32, tag="nrm")
            rn_all = persist.tile([128, MT], f32, tag="rn")
            ms_all = persist.tile([128, MT], f32, tag="ms")
            sqd = persist.tile([128, HW], bf16, tag="sqd")
            xw_pool = ctx.enter_context(tc.tile_pool(name="xw", bufs=4))
            dn_pool = ctx.enter_context(tc.tile_pool(name="dn", bufs=2))
            stats = ctx.enter_context(tc.tile_pool(name="stats", bufs=6))
            xs_pool = ctx.enter_context(tc.tile_pool(name="xs", bufs=2))
            row_pool = ctx.enter_context(tc.tile_pool(name="rows", bufs=2))
            scr_pool = ctx.enter_context(tc.tile_pool(name="scr", bufs=2))

            ps_pool = ctx.enter_context(
                tc.tile_pool(name="ps", bufs=2, space="PSUM")
            )
            psg_pool = ctx.enter_context(
                tc.tile_pool(name="psg", bufs=2, space="PSUM")
            )
            pm_pool = ctx.enter_context(
                tc.tile_pool(name="pm", bufs=1, space="PSUM")
            )
            pt_pool = ctx.enter_context(
                tc.tile_pool(name="pt", bufs=1, space="PSUM")
            )

            if repeat > 1:
                ctx.enter_context(tc.For_i(0, repeat, 1))

            def stage0_load(m):
                xw = xw_pool.tile([128, HW], bf16, tag="xw")
                nc.sync.dma_start(out=xw, in_=x1s[m * 128:(m + 1) * 128, :])
                nc.scalar.activation(
                    out=sqd, in_=xw, func=AF.Square,
                    accum_out=sumsq_all[:, m:m + 1],
                )

            def stage0_sum(m):
                xw = xw_pool.tile([128, HW], bf16, tag="xw")
                nc.sync.dma_start(out=xw, in_=x1s[m * 128:(m + 1) * 128, :])
                nc.scalar.activation(
                    out=sqd, in_=xw, func=AF.Copy,
                    accum_out=s1_all[:, m:m + 1],
                )

            def stage0_stats():
                # one batched op per stat over all 32 m-tiles
                nc.scalar.activation(out=nrm_all, in_=sumsq_all, func=AF.Sqrt)
                nc.vector.reciprocal(out=rn_all, in_=nrm_all)
                nc.vector.tensor_scalar(
                    out=ms_all, in0=s1_all, scalar1=1.0 / HW, scalar2=None,
                    op0=OP.mult,
                )
                mq_all = stats.tile([128, MT], f32, tag="mq")
                nc.vector.tensor_tensor(
                    out=mq_all, in0=ms_all, in1=rn_all, op=OP.mult
                )
                msq_all = stats.tile([128, MT], f32, tag="msq")
                nc.vector.tensor_tensor(
                    out=msq_all, in0=mq_all, in1=mq_all, op=OP.mult
                )
                nc.vector.tensor_scalar(
                    out=base_t, in0=msq_all, scalar1=-float(HW), scalar2=1.0,
                    op0=OP.mult, op1=OP.add,
                )

            def stage0_m(m):
                xw = xw_pool.tile([128, HW], bf16, tag="xw")
                nc.sync.dma_start(out=xw, in_=x1s[m * 128:(m + 1) * 128, :])
                dn = dn_pool.tile([128, HW], f32r, tag="dn")
                nc.vector.tensor_scalar(
                    out=dn, in0=xw, scalar1=ms_all[:, m:m + 1],
                    scalar2=rn_all[:, m:m + 1],
                    op0=OP.subtract, op1=OP.mult,
                )
                nc.vector.tensor_copy(out=d_res[:, m, 0, :], in_=dn[:, 0:392])
                nc.vector.tensor_copy(out=d_res[:, m, 1, :], in_=dn[:, 392:784])
                for kt, (koff, klen) in enumerate(KT):
                    pt = pt_pool.tile([128, 128], f32r, tag="pt")
                    nc.tensor.transpose(
                        out=pt[:klen, :], in_=dn[:, koff:koff + klen],
                        identity=ident,
                    )
                    dst = dtT[:klen, m, kt, :]
                    if kt % 2 == 0:
                        nc.scalar.mul(out=dst, in_=pt[:klen, :], mul=SD)
                    else:
                        nc.vector.tensor_scalar(
                            out=dst, in0=pt[:klen, :], scalar1=SD,
                            scalar2=None, op0=OP.mult,
                        )

            def gram_class(n):
                xs = xs_pool.tile([128, RTN, HW], fp8, tag="xs")
                for rt in range(RTN):
                    nc.sync.dma_start(out=xs[:, rt, :], in_=x2d[n, rt, :, :])
                srow = row_pool.tile([1, HW], fp8, tag="srow")
                ssrow = row_pool.tile([1, HW], fp8, tag="ssrow")
                for qi, (qoff, qlen) in enumerate(QT):
                    pm = pm_pool.tile([1, 512], f32, tag="pm")
                    for r in range(RTN // 2):
                        nc.tensor.matmul(
                            pm[:1, :qlen],
                            lhsT=ones2[:, :, 0:1],
                            rhs=xs[:, 2 * r:2 * r + 2, qoff:qoff + qlen],
                            start=(r == 0), stop=(r == RTN // 2 - 1),
                            perf_mode=DRM,
                        )
                    qs = slice(qoff, qoff + qlen)
                    nc.scalar.mul(out=srow[:, qs], in_=pm[:1, :qlen], mul=1.0)
                    nc.scalar.mul(
                        out=ssrow[:, qs], in_=pm[:1, :qlen], mul=-1.0 / NR
                    )
                for mc, (mcoff, mclen) in enumerate(KT):
                    for qi, (qoff, qlen) in enumerate(QT):
                        psg = psg_pool.tile([128, 512], f32, tag="psg")
                        for r in range(RTN // 2):
                            nc.tensor.matmul(
                                psg[:mclen, :qlen],
                                lhsT=xs[:, 2 * r:2 * r + 2,
                                        mcoff:mcoff + mclen],
                                rhs=xs[:, 2 * r:2 * r + 2, qoff:qoff + qlen],
                                start=(r == 0), stop=False,
                                perf_mode=DRM,
                            )
                        has_diag = (mcoff < qoff + qlen
                                    and qoff < mcoff + mclen)
                        nc.tensor.matmul(
                            psg[:mclen, :qlen],
                            lhsT=ssrow[:1, mcoff:mcoff + mclen],
                            rhs=srow[:1, qoff:qoff + qlen],
                            start=False, stop=not has_diag,
                            skip_group_check=True,
                        )
                        if has_diag:
                            s_off = EYE_OFF - mcoff + qoff
                            nc.tensor.matmul(
                                psg[:mclen, :qlen],
                                lhsT=ai[:, :mclen],
                                rhs=eye[:, s_off:s_off + qlen],
                                start=False, stop=True,
                                skip_group_check=True,
                            )
                        dst = cov[:mclen, n, mc, qoff:qoff + qlen]
                        if qi == 0:
                            nc.vector.tensor_scalar(
                                out=dst, in0=psg[:mclen, :qlen],
                                scalar1=SC / (NR - 1), scalar2=None, op0=OP.mult,
                            )
                        else:
                            nc.scalar.mul(
                                out=dst, in_=psg[:mclen, :qlen],
                                mul=SC / (NR - 1),
                            )

            # ---- Phase A: batched stage-0 with gram classes interleaved ----
            for i in range(MT):
                stage0_load(i)
                if i >= MT - 2 - NW and i < MT - 2:
                    gram_class(i - (MT - 2 - NW))
            for i in range(MT):
                stage0_sum(i)
            stage0_stats()
            for i in range(MT):
                stage0_m(i)

            # ---- Phase B: sim = (D @ V) . D row-reduced ----
            for n in range(NW if abl != "nophaseb" else 0):
                for m in range(MT):
                    ps = ps_pool.tile([128, 2, 512], f32, tag="ps")
                    for qi, (qoff, qlen) in enumerate(QT):
                        if abl == "nomm":
                            break
                        for t in range(NKT8 // 2):
                            nc.tensor.matmul(
                                ps[:, qi, :qlen],
                                lhsT=dtT[:, m, 2 * t:2 * t + 2, :],
                                rhs=cov[:, n, 2 * t:2 * t + 2,
                                        qoff:qoff + qlen],
                                start=(t == 0), stop=(t == NKT8 // 2 - 1),
                                perf_mode=DRM,
                            )
                    if abl == "nostt":
                        continue
                    scr = scr_pool.tile([128, 2, 392], bf16, tag="scr")
                    if m % 2 == 1:
                        # ACT drains PSUM (descale fused); DVE multiplies
                        # from SBUF — splits the reduce across two engines
                        sc = scr_pool.tile([128, 2, 392], bf16, tag="sc")
                        nc.scalar.mul(
                            out=sc, in_=ps[:, :, :392], mul=1.0 / (SD * SC)
                        )
                        nc.vector.scalar_tensor_tensor(
                            out=scr, in0=sc, scalar=1.0,
                            in1=d_res[:, m, :, :],
                            op0=OP.mult, op1=OP.mult,
                            accum_out=out_acc[:, m, n:n + 1],
                        )
                    else:
                        nc.vector.scalar_tensor_tensor(
                            out=scr,
                            in0=ps[:, :, :392],
                            scalar=1.0 / (SD * SC),
                            in1=d_res[:, m, :, :],
                            op0=OP.mult, op1=OP.mult,
                            accum_out=out_acc[:, m, n:n + 1],
                        )

            for m in range(MT):
                nc.vector.tensor_scalar(
                    out=out_acc[:, m, :], in0=out_acc[:, m, :],
                    scalar1=base_t[:, m:m + 1], scalar2=None, op0=OP.add,
                )
                nc.sync.dma_start(out=out[m], in_=out_acc[:, m, :])
            if debug:
                nc.sync.dma_start(out=dbg_dtT[:, :, :, :], in_=dtT)
                nc.sync.dma_start(out=dbg_cov[:, :, :, :], in_=cov)
                nc.sync.dma_start(out=dbg_base[:, :], in_=base_t)
                nc.sync.dma_start(out=dbg_dres[:, :, :, :], in_=d_res)

    nc.finalize()
    return nc


def get_program():
    key = "nc"
    if key not in _STATE:
        _STATE[key] = _build_program()
    return _STATE[key]


def make_in_maps(x1, x2):
    import ml_dtypes

    x1f = np.asarray(x1, dtype=np.float32).reshape(B * C, HW)
    x1b = np.ascontiguousarray(x1f).astype(ml_dtypes.bfloat16)
    x2f = np.asarray(x2, dtype=np.float32).reshape(NW, RTN, 128, HW)
    x2q = np.ascontiguousarray(x2f).astype(ml_dtypes.float8_e4m3)
    return [
        {"x1s": x1b[c * NI:(c + 1) * NI], "x2": x2q}
        for c in range(N_CORES)
    ]


def assemble_output(core_outs):
    # per-core (MT, 128, NW) -> (BSH, NW*C); concat over cores -> (B, NW*C)
    parts = [
        o.reshape(NI, NW).reshape(BSH, C, NW).transpose(0, 2, 1)
        .reshape(BSH, NW * C)
        for o in core_outs
    ]
    return np.ascontiguousarray(np.concatenate(parts, axis=0), dtype=np.float32)


def kernel(x1, x2):
    from concourse.bass_utils import run_bass_kernel_spmd

    nc = get_program()
    in_maps = make_in_maps(x1, x2)
    res = run_bass_kernel_spmd(nc, in_maps, list(range(N_CORES)))
    return assemble_output([res.results[i]["out"] for i in range(N_CORES)])


# revision 15
# speedup vs baseline: 1.0175x; 1.0175x over previous
"""ChannelCovarianceBlock Trainium2 kernel (fp8 DoubleRow version).

Computes, for queries x1 (B, C, h, w) and support sets x2 (nw, Bs, C, h, w):
  cov_n = Cov(x2[n].reshape(Bs*C, hw))            (hw, hw) per class
  d     = normalize-and-center rows of x1.reshape(B*C, hw)
  sim[b, n, c] = d[bc] @ cov_n @ d[bc]^T          -> (B, nw*C)

Sharding: data-parallel over B across 8 NeuronCores (32 queries each);
each core computes all 10 class covariances from the full x2 (redundant
but collective-free) using the Gram identity cov = (X^T X - s s^T/N)/(N-1).

Numerics: matmuls run in fp8e4 (e4m3) with MatmulPerfMode.DoubleRow
(0.5 PE cycles/row, 2x bf16 throughput). To survive fp8's 3-bit
mantissa, the covariance is split as cov = I + V: the exact base term
||d||^2 = 1 - hw*m^2 (m = row mean of the normalized query) is computed
from stage-0 stats in f32, and only the small-valued V = cov - I is
quantized to fp8 (the I subtraction happens inside PSUM via an exact
f32r matmul against a shifted-identity tile). d is scaled by 16 and V
by 16 before fp8 quantization; the 1/256 descale folds into the final
multiply-reduce. Measured rel err ~8e-3 (numpy model 7.8e-3).

Per-core dataflow (single pass, no DRAM spills):
  phase A (interleaved per m-tile / per class to keep PE busy):
    stage 0: load x1 rows (bf16), compute norm/mean stats, write the
             normalized d as bf16 (d_res, stt operand) and build D^T in
             fp8 via PE transposes (dtT, matmul lhsT), both SBUF-resident.
    gram:    per class, 5 DoubleRow Gram matmuls per (k-block, q-chunk)
             + rank-1 mean correction + f32r -(NR-1)*I matmul, then
             copy PSUM -> V fp8 (all 10 classes stay resident).
  phase B: per (class, m-tile): 8 uniform DoubleRow matmuls (k-pairs
           (0,1)(2,3)(4,5)(6,7), kt=7 zero pad; a plain 16-partition
           remainder matmul measures ~1 us each on HW - avoid) ->
           S = D @ V in PSUM, then one DVE scalar_tensor_tensor
           (S * d_res, accum_out) -> out_acc += base, DMA out.

Measured on 8 trn2 cores: 915 us/exec, rel err 7.76e-3 (vs 1,835 us
f32r baseline). Ablations: phase A ~280 us, phase-B matmul stream
~507 us (each 392-row DR matmul ~198 ns incl. weight load; walrus
runs with --enable-ldw-opt=false so weight loads are never amortized),
stt adds ~130-230 us non-overlapped.
"""

import os
import sys

for _p in ("/opt/trn_rl_repo", "/root/.axon_site/_ro/trn_rl_repo"):
    if os.path.isdir(_p) and _p not in sys.path:
        sys.path.append(_p)

import numpy as np

# Problem constants (hardcoded per spec).
B, C, H, W = 256, 128, 28, 28
NW, BS = 10, 10
HW = H * W            # 784
N_CORES = 8
BSH = B // N_CORES    # 32 queries per core
NI = BSH * C          # 4096 rows per core
NR = BS * C           # 1280 support rows per class
RTN = NR // 128       # 10 row-tiles per class

# K-tiles over the hw contraction dim (partition dim <= 128).
KT = [(k * 128, min(128, HW - k * 128)) for k in range((HW + 127) // 128)]
NKT = len(KT)         # 7 (6 full + 16-row remainder)
NKT8 = 8              # k-tile slots incl. zero pad so kt (6,7) forms a DR pair
NDR = 3               # DoubleRow k-tile pairs (0,1)(2,3)(4,5); kt=6 plain
QT = [(0, 392), (392, 392)]
MT = NI // 128        # 32 i-tiles per core

SD = 16.0             # d scale before fp8 quantization
SC = 16.0             # V scale before fp8 quantization
EYE_OFF = 384         # identity block column offset in the EYE tile

_STATE = {}


def _build_program(repeat=None, abl=None):
    if repeat is None:
        repeat = int(os.environ.get("CCB_REPEAT", "1"))
    if abl is None:
        abl = os.environ.get("CCB_ABL", "full")
    import concourse.bass as bass
    import concourse.bacc as bacc
    import concourse.tile as tile
    from concourse import mybir
    from concourse.masks import make_identity
    from contextlib import ExitStack

    f32 = mybir.dt.float32
    f32r = mybir.dt.float32r
    bf16 = mybir.dt.bfloat16
    fp8 = mybir.dt.float8e4
    DRM = mybir.MatmulPerfMode.DoubleRow
    ALPHA = float(np.sqrt(NR - 1.0))

    nc = bacc.Bacc()
    x1s = nc.declare_dram_parameter("x1s", [NI, HW], bf16, isOutput=False)
    x2d = nc.declare_dram_parameter("x2", [NW, RTN, 128, HW], fp8, isOutput=False)
    out = nc.declare_dram_parameter("out", [MT, 128, NW], f32, isOutput=True)
    debug = os.environ.get("CCB_DEBUG") == "1"
    if debug:
        dbg_dtT = nc.declare_dram_parameter(
            "dbg_dtT", [128, MT, NKT8, 128], fp8, isOutput=True)
        dbg_cov = nc.declare_dram_parameter(
            "dbg_cov", [128, NW, NKT8, HW], fp8, isOutput=True)
        dbg_base = nc.declare_dram_parameter(
            "dbg_base", [128, MT], f32, isOutput=True)
        dbg_dres = nc.declare_dram_parameter(
            "dbg_dres", [128, MT, 2, 392], bf16, isOutput=True)

    AF = mybir.ActivationFunctionType
    OP = mybir.AluOpType

    with tile.TileContext(nc) as tc:
        with ExitStack() as ctx:
            persist = ctx.enter_context(tc.tile_pool(name="persist", bufs=1))
            ident_f = persist.tile([128, 128], f32, tag="ident_f")
            make_identity(nc, ident_f)
            # f32r copies must come from walrus-approved producers (DVE).
            ident = persist.tile([128, 128], f32r, tag="ident")
            nc.vector.tensor_copy(out=ident, in_=ident_f)
            # AI = +alpha*I, EYE carries -alpha at [p, EYE_OFF+p]; their
            # product in PSUM subtracts (NR-1)*I from the Gram exactly.
            ai = persist.tile([128, 128], f32r, tag="ai")
            nc.vector.tensor_scalar(
                out=ai, in0=ident_f, scalar1=ALPHA, scalar2=None, op0=OP.mult
            )
            eye_f = persist.tile([128, HW], f32, tag="eye_f")
            nc.vector.memset(eye_f, 0.0)
            nc.vector.tensor_scalar(
                out=eye_f[:, EYE_OFF:EYE_OFF + 128], in0=ident_f,
                scalar1=-ALPHA, scalar2=None, op0=OP.mult,
            )
            eye = persist.tile([128, HW], f32r, tag="eye")
            nc.vector.tensor_copy(out=eye, in_=eye_f)
            # DR weight APs need even, 16B-aligned outer free steps.
            ones2 = persist.tile([128, 2, 16], fp8, tag="ones2")
            nc.vector.memset(ones2, 1.0)
            # stt operand: normalized d, bf16, (2, 392) layout to match psum
            d_res = persist.tile([128, MT, 2, 392], bf16, tag="d_res")
            # matmul lhsT: D^T in fp8, scaled by SD
            dtT = persist.tile([128, MT, NKT8, 128], fp8, tag="dtT")
            # all 10 class V matrices, fp8, scaled by SC
            cov = persist.tile([128, NW, NKT8, HW], fp8, tag="cov")
            # zero the DR pad: kt=7 plane and partitions 16.. of kt=6
            nc.vector.memset(dtT[:, :, NKT8 - 1, :], 0.0)
            nc.vector.memset(dtT[:, :, NKT - 1, :], 0.0)
            nc.vector.memset(cov[:, :, NKT8 - 1, :], 0.0)
            nc.vector.memset(cov[:, :, NKT - 1, :], 0.0)
            out_acc = persist.tile([128, MT, NW], f32, tag="out_acc")
            base_t = persist.tile([128, MT], f32, tag="base")

            sumsq_all = persist.tile([128, MT], f32, tag="sumsq")
            s1_all = persist.tile([128, MT], f32, tag="s1")
            nrm_all = persist.tile([128, MT], f32, tag="nrm")
            rn_all = persist.tile([128, MT], f32, tag="rn")
            ms_all = persist.tile([128, MT], f32, tag="ms")
            sqd = persist.tile([128, HW], bf16, tag="sqd")
            xw_pool = ctx.enter_context(tc.tile_pool(name="xw", bufs=4))
            dn_pool = ctx.enter_context(tc.tile_pool(name="dn", bufs=2))
            stats = ctx.enter_context(tc.tile_pool(name="stats", bufs=6))
            xs_pool = ctx.enter_context(tc.tile_pool(name="xs", bufs=2))
            row_pool = ctx.enter_context(tc.tile_pool(name="rows", bufs=2))
            scr_pool = ctx.enter_context(tc.tile_pool(name="scr", bufs=2))

            ps_pool = ctx.enter_context(
                tc.tile_pool(name="ps", bufs=2, space="PSUM")
            )
            psg_pool = ctx.enter_context(
                tc.tile_pool(name="psg", bufs=2, space="PSUM")
            )
            pm_pool = ctx.enter_context(
                tc.tile_pool(name="pm", bufs=1, space="PSUM")
            )
            pt_pool = ctx.enter_context(
                tc.tile_pool(name="pt", bufs=1, space="PSUM")
            )

            if repeat > 1:
                ctx.enter_context(tc.For_i(0, repeat, 1))

            def stage0_load(m):
                xw = xw_pool.tile([128, HW], bf16, tag="xw")
                nc.sync.dma_start(out=xw, in_=x1s[m * 128:(m + 1) * 128, :])
                nc.scalar.activation(
                    out=sqd, in_=xw, func=AF.Square,
                    accum_out=sumsq_all[:, m:m + 1],
                )

            def stage0_sum(m):
                xw = xw_pool.tile([128, HW], bf16, tag="xw")
                nc.sync.dma_start(out=xw, in_=x1s[m * 128:(m + 1) * 128, :])
                nc.scalar.activation(
                    out=sqd, in_=xw, func=AF.Copy,
                    accum_out=s1_all[:, m:m + 1],
                )

            def stage0_stats():
                # one batched op per stat over all 32 m-tiles
                nc.scalar.activation(out=nrm_all, in_=sumsq_all, func=AF.Sqrt)
                nc.vector.reciprocal(out=rn_all, in_=nrm_all)
                nc.vector.tensor_scalar(
                    out=ms_all, in0=s1_all, scalar1=1.0 / HW, scalar2=None,
                    op0=OP.mult,
                )
                mq_all = stats.tile([128, MT], f32, tag="mq")
                nc.vector.tensor_tensor(
                    out=mq_all, in0=ms_all, in1=rn_all, op=OP.mult
                )
                msq_all = stats.tile([128, MT], f32, tag="msq")
                nc.vector.tensor_tensor(
                    out=msq_all, in0=mq_all, in1=mq_all, op=OP.mult
                )
                nc.vector.tensor_scalar(
                    out=base_t, in0=msq_all, scalar1=-float(HW), scalar2=1.0,
                    op0=OP.mult, op1=OP.add,
                )

            def stage0_m(m):
                xw = xw_pool.tile([128, HW], bf16, tag="xw")
                nc.sync.dma_start(out=xw, in_=x1s[m * 128:(m + 1) * 128, :])
                dn = dn_pool.tile([128, HW], f32r, tag="dn")
                nc.vector.tensor_scalar(
                    out=dn, in0=xw, scalar1=ms_all[:, m:m + 1],
                    scalar2=rn_all[:, m:m + 1],
                    op0=OP.subtract, op1=OP.mult,
                )
                nc.vector.tensor_copy(out=d_res[:, m, 0, :], in_=dn[:, 0:392])
                nc.vector.tensor_copy(out=d_res[:, m, 1, :], in_=dn[:, 392:784])
                for kt, (koff, klen) in enumerate(KT):
                    pt = pt_pool.tile([128, 128], f32r, tag="pt")
                    nc.tensor.transpose(
                        out=pt[:klen, :], in_=dn[:, koff:koff + klen],
                        identity=ident,
                    )
                    dst = dtT[:klen, m, kt, :]
                    if kt % 2 == 0:
                        nc.scalar.mul(out=dst, in_=pt[:klen, :], mul=SD)
                    else:
                        nc.vector.tensor_scalar(
                            out=dst, in0=pt[:klen, :], scalar1=SD,
                            scalar2=None, op0=OP.mult,
                        )

            def gram_class(n):
                xs = xs_pool.tile([128, RTN, HW], fp8, tag="xs")
                for rt in range(RTN):
                    nc.sync.dma_start(out=xs[:, rt, :], in_=x2d[n, rt, :, :])
                srow = row_pool.tile([1, HW], fp8, tag="srow")
                ssrow = row_pool.tile([1, HW], fp8, tag="ssrow")
                for qi, (qoff, qlen) in enumerate(QT):
                    pm = pm_pool.tile([1, 512], f32, tag="pm")
                    for r in range(RTN // 2):
                        nc.tensor.matmul(
                            pm[:1, :qlen],
                            lhsT=ones2[:, :, 0:1],
                            rhs=xs[:, 2 * r:2 * r + 2, qoff:qoff + qlen],
                            start=(r == 0), stop=(r == RTN // 2 - 1),
                            perf_mode=DRM,
                        )
                    qs = slice(qoff, qoff + qlen)
                    nc.scalar.mul(out=srow[:, qs], in_=pm[:1, :qlen], mul=1.0)
                    nc.scalar.mul(
                        out=ssrow[:, qs], in_=pm[:1, :qlen], mul=-1.0 / NR
                    )
                for mc, (mcoff, mclen) in enumerate(KT):
                    for qi, (qoff, qlen) in enumerate(QT):
                        psg = psg_pool.tile([128, 512], f32, tag="psg")
                        for r in range(RTN // 2):
                            nc.tensor.matmul(
                                psg[:mclen, :qlen],
                                lhsT=xs[:, 2 * r:2 * r + 2,
                                        mcoff:mcoff + mclen],
                                rhs=xs[:, 2 * r:2 * r + 2, qoff:qoff + qlen],
                                start=(r == 0), stop=False,
                                perf_mode=DRM,
                            )
                        has_diag = (mcoff < qoff + qlen
                                    and qoff < mcoff + mclen)
                        nc.tensor.matmul(
                            psg[:mclen, :qlen],
                            lhsT=ssrow[:1, mcoff:mcoff + mclen],
                            rhs=srow[:1, qoff:qoff + qlen],
                            start=False, stop=not has_diag,
                            skip_group_check=True,
                        )
                        if has_diag:
                            s_off = EYE_OFF - mcoff + qoff
                            nc.tensor.matmul(
                                psg[:mclen, :qlen],
                                lhsT=ai[:, :mclen],
                                rhs=eye[:, s_off:s_off + qlen],
                                start=False, stop=True,
                                skip_group_check=True,
                            )
                        dst = cov[:mclen, n, mc, qoff:qoff + qlen]
                        if qi == 0:
                            nc.vector.tensor_scalar(
                                out=dst, in0=psg[:mclen, :qlen],
                                scalar1=SC / (NR - 1), scalar2=None, op0=OP.mult,
                            )
                        else:
                            nc.scalar.mul(
                                out=dst, in_=psg[:mclen, :qlen],
                                mul=SC / (NR - 1),
                            )

            # ---- Phase A: batched stage-0 with gram classes interleaved ----
            for i in range(MT):
                stage0_load(i)
                if i >= MT - 2 - NW and i < MT - 2:
                    gram_class(i - (MT - 2 - NW))
            for i in range(MT):
                stage0_sum(i)
            stage0_stats()
            for i in range(MT):
                stage0_m(i)

            # ---- Phase B: sim = (D @ V) . D row-reduced ----
            for n in range(NW if abl != "nophaseb" else 0):
                for m in range(MT):
                    ps = ps_pool.tile([128, 2, 512], f32, tag="ps")
                    for qi, (qoff, qlen) in enumerate(QT):
                        if abl == "nomm":
                            break
                        for t in range(NKT8 // 2):
                            nc.tensor.matmul(
                                ps[:, qi, :qlen],
                                lhsT=dtT[:, m, 2 * t:2 * t + 2, :],
                                rhs=cov[:, n, 2 * t:2 * t + 2,
                                        qoff:qoff + qlen],
                                start=(t == 0), stop=(t == NKT8 // 2 - 1),
                                perf_mode=DRM,
                            )
                    if abl == "nostt":
                        continue
                    scr = scr_pool.tile([128, 2, 392], bf16, tag="scr")
                    nc.vector.scalar_tensor_tensor(
                        out=scr,
                        in0=ps[:, :, :392],
                        scalar=1.0 / (SD * SC),
                        in1=d_res[:, m, :, :],
                        op0=OP.mult, op1=OP.mult,
                        accum_out=out_acc[:, m, n:n + 1],
                    )

            for m in range(MT):
                nc.vector.tensor_scalar(
                    out=out_acc[:, m, :], in0=out_acc[:, m, :],
                    scalar1=base_t[:, m:m + 1], scalar2=None, op0=OP.add,
                )
                nc.sync.dma_start(out=out[m], in_=out_acc[:, m, :])
            if debug:
                nc.sync.dma_start(out=dbg_dtT[:, :, :, :], in_=dtT)
                nc.sync.dma_start(out=dbg_cov[:, :, :, :], in_=cov)
                nc.sync.dma_start(out=dbg_base[:, :], in_=base_t)
                nc.sync.dma_start(out=dbg_dres[:, :, :, :], in_=d_res)

    nc.finalize()
    return nc


def get_program():
    key = "nc"
    if key not in _STATE:
        _STATE[key] = _build_program()
    return _STATE[key]


def make_in_maps(x1, x2):
    import ml_dtypes

    x1f = np.asarray(x1, dtype=np.float32).reshape(B * C, HW)
    x1b = np.ascontiguousarray(x1f).astype(ml_dtypes.bfloat16)
    x2f = np.asarray(x2, dtype=np.float32).reshape(NW, RTN, 128, HW)
    x2q = np.ascontiguousarray(x2f).astype(ml_dtypes.float8_e4m3)
    return [
        {"x1s": x1b[c * NI:(c + 1) * NI], "x2": x2q}
        for c in range(N_CORES)
    ]


def assemble_output(core_outs):
    # per-core (MT, 128, NW) -> (BSH, NW*C); concat over cores -> (B, NW*C)
    parts = [
        o.reshape(NI, NW).reshape(BSH, C, NW).transpose(0, 2, 1)
        .reshape(BSH, NW * C)
        for o in core_outs
    ]
    return np.ascontiguousarray(np.concatenate(parts, axis=0), dtype=np.float32)


def kernel(x1, x2):
    from concourse.bass_utils import run_bass_kernel_spmd

    nc = get_program()
    in_maps = make_in_maps(x1, x2)
    res = run_bass_kernel_spmd(nc, in_maps, list(range(N_CORES)))
    return assemble_output([res.results[i]["out"] for i in range(N_CORES)])


# revision 17
# speedup vs baseline: 1.0652x; 1.0469x over previous
"""ChannelCovarianceBlock Trainium2 kernel (fp8 DoubleRow version).

Computes, for queries x1 (B, C, h, w) and support sets x2 (nw, Bs, C, h, w):
  cov_n = Cov(x2[n].reshape(Bs*C, hw))            (hw, hw) per class
  d     = normalize-and-center rows of x1.reshape(B*C, hw)
  sim[b, n, c] = d[bc] @ cov_n @ d[bc]^T          -> (B, nw*C)

Sharding: data-parallel over B across 8 NeuronCores (32 queries each);
each core computes all 10 class covariances from the full x2 (redundant
but collective-free) using the Gram identity cov = (X^T X - s s^T/N)/(N-1).

Numerics: matmuls run in fp8e4 (e4m3) with MatmulPerfMode.DoubleRow
(0.5 PE cycles/row, 2x bf16 throughput). To survive fp8's 3-bit
mantissa, the covariance is split as cov = I + V: the exact base term
||d||^2 = 1 - hw*m^2 (m = row mean of the normalized query) is computed
from stage-0 stats in f32, and only the small-valued V = cov - I is
quantized to fp8 (the I subtraction happens inside PSUM via an exact
f32r matmul against a shifted-identity tile). d is scaled by 16 and V
by 16 before fp8 quantization; the 1/256 descale folds into the final
multiply-reduce. Measured rel err ~8e-3 (numpy model 7.8e-3).

Per-core dataflow (single pass, no DRAM spills):
  phase A (interleaved per m-tile / per class to keep PE busy):
    stage 0: load x1 rows (bf16), compute norm/mean stats, write the
             normalized d as bf16 (d_res, stt operand) and build D^T in
             fp8 via PE transposes (dtT, matmul lhsT), both SBUF-resident.
    gram:    per class, 5 DoubleRow Gram matmuls per (k-block, q-chunk)
             + rank-1 mean correction + f32r -(NR-1)*I matmul, then
             copy PSUM -> V fp8 (all 10 classes stay resident).
  phase B: per (class, m-tile): 8 uniform DoubleRow matmuls (k-pairs
           (0,1)(2,3)(4,5)(6,7), kt=7 zero pad; a plain 16-partition
           remainder matmul measures ~1 us each on HW - avoid) ->
           S = D @ V in PSUM, then one DVE scalar_tensor_tensor
           (S * d_res, accum_out) -> out_acc += base, DMA out.

Measured on 8 trn2 cores: 915 us/exec, rel err 7.76e-3 (vs 1,835 us
f32r baseline). Ablations: phase A ~280 us, phase-B matmul stream
~507 us (each 392-row DR matmul ~198 ns incl. weight load; walrus
runs with --enable-ldw-opt=false so weight loads are never amortized),
stt adds ~130-230 us non-overlapped.
"""

import os
import sys

for _p in ("/opt/trn_rl_repo", "/root/.axon_site/_ro/trn_rl_repo"):
    if os.path.isdir(_p) and _p not in sys.path:
        sys.path.append(_p)

import numpy as np

# Problem constants (hardcoded per spec).
B, C, H, W = 256, 128, 28, 28
NW, BS = 10, 10
HW = H * W            # 784
N_CORES = 8
BSH = B // N_CORES    # 32 queries per core
NI = BSH * C          # 4096 rows per core
NR = BS * C           # 1280 support rows per class
RTN = NR // 128       # 10 row-tiles per class

# K-tiles over the hw contraction dim (partition dim <= 128).
KT = [(k * 128, min(128, HW - k * 128)) for k in range((HW + 127) // 128)]
NKT = len(KT)         # 7 (6 full + 16-row remainder)
NKT8 = 8              # k-tile slots incl. zero pad so kt (6,7) forms a DR pair
NDR = 3               # DoubleRow k-tile pairs (0,1)(2,3)(4,5); kt=6 plain
QT = [(0, 392), (392, 392)]
MT = NI // 128        # 32 i-tiles per core

SD = 16.0             # d scale before fp8 quantization
SC = 16.0             # V scale before fp8 quantization
EYE_OFF = 384         # identity block column offset in the EYE tile

_STATE = {}


def _build_program(repeat=None, abl=None):
    if repeat is None:
        repeat = int(os.environ.get("CCB_REPEAT", "1"))
    if abl is None:
        abl = os.environ.get("CCB_ABL", "full")
    import concourse.bass as bass
    import concourse.bacc as bacc
    import concourse.tile as tile
    from concourse import mybir
    from concourse.masks import make_identity
    from contextlib import ExitStack

    f32 = mybir.dt.float32
    f32r = mybir.dt.float32r
    bf16 = mybir.dt.bfloat16
    fp8 = mybir.dt.float8e4
    DRM = mybir.MatmulPerfMode.DoubleRow
    ALPHA = float(np.sqrt(NR - 1.0))

    nc = bacc.Bacc()
    x1s = nc.declare_dram_parameter("x1s", [NI, HW], bf16, isOutput=False)
    x2d = nc.declare_dram_parameter("x2", [NW, RTN, 128, HW], fp8, isOutput=False)
    out = nc.declare_dram_parameter("out", [MT, 128, NW], f32, isOutput=True)
    debug = os.environ.get("CCB_DEBUG") == "1"
    if debug:
        dbg_dtT = nc.declare_dram_parameter(
            "dbg_dtT", [128, MT, NKT8, 128], fp8, isOutput=True)
        dbg_cov = nc.declare_dram_parameter(
            "dbg_cov", [128, NW, NKT8, HW], fp8, isOutput=True)
        dbg_base = nc.declare_dram_parameter(
            "dbg_base", [128, MT], f32, isOutput=True)
        dbg_dres = nc.declare_dram_parameter(
            "dbg_dres", [128, MT, 2, 392], bf16, isOutput=True)

    AF = mybir.ActivationFunctionType
    OP = mybir.AluOpType

    with tile.TileContext(nc) as tc:
        with ExitStack() as ctx:
            persist = ctx.enter_context(tc.tile_pool(name="persist", bufs=1))
            ident_f = persist.tile([128, 128], f32, tag="ident_f")
            make_identity(nc, ident_f)
            # f32r copies must come from walrus-approved producers (DVE).
            ident = persist.tile([128, 128], f32r, tag="ident")
            nc.vector.tensor_copy(out=ident, in_=ident_f)
            # AI = +alpha*I, EYE carries -alpha at [p, EYE_OFF+p]; their
            # product in PSUM subtracts (NR-1)*I from the Gram exactly.
            ai = persist.tile([128, 128], f32r, tag="ai")
            nc.vector.tensor_scalar(
                out=ai, in0=ident_f, scalar1=ALPHA, scalar2=None, op0=OP.mult
            )
            eye_f = persist.tile([128, HW], f32, tag="eye_f")
            nc.vector.memset(eye_f, 0.0)
            nc.vector.tensor_scalar(
                out=eye_f[:, EYE_OFF:EYE_OFF + 128], in0=ident_f,
                scalar1=-ALPHA, scalar2=None, op0=OP.mult,
            )
            eye = persist.tile([128, HW], f32r, tag="eye")
            nc.vector.tensor_copy(out=eye, in_=eye_f)
            # symmetry fold: W = mask (x) V with mask 2/1/0 above/on/below
            # the diagonal; TRI slices address any (k-block, q-chunk) block
            TRI_W, TRI_OFF = 1552, 768
            tri = persist.tile([128, TRI_W], f32, tag="tri")
            nc.vector.memset(tri, 2.0)
            nc.gpsimd.affine_select(
                out=tri, in_=tri, compare_op=OP.is_ge, fill=0.0,
                base=-TRI_OFF, pattern=[[1, TRI_W]], channel_multiplier=-1,
            )
            nc.vector.tensor_tensor(
                out=tri[:, TRI_OFF:TRI_OFF + 128],
                in0=tri[:, TRI_OFF:TRI_OFF + 128],
                in1=ident_f, op=OP.subtract,
            )
            # DR weight APs need even, 16B-aligned outer free steps.
            ones2 = persist.tile([128, 2, 16], fp8, tag="ones2")
            nc.vector.memset(ones2, 1.0)
            # stt operand: normalized d, bf16, (2, 392) layout to match psum
            d_res = persist.tile([128, MT, 2, 392], bf16, tag="d_res")
            # matmul lhsT: D^T in fp8, scaled by SD
            dtT = persist.tile([128, MT, NKT8, 128], fp8, tag="dtT")
            # all 10 class V matrices, fp8, scaled by SC
            cov = persist.tile([128, NW, NKT8, HW], fp8, tag="cov")
            # zero the DR pad: kt=7 plane and partitions 16.. of kt=6
            nc.vector.memset(dtT[:, :, NKT8 - 1, :], 0.0)
            nc.vector.memset(dtT[:, :, NKT - 1, :], 0.0)
            nc.vector.memset(cov[:, :, NKT8 - 1, :], 0.0)
            nc.vector.memset(cov[:, :, NKT - 1, :], 0.0)
            out_acc = persist.tile([128, MT, NW], f32, tag="out_acc")
            base_t = persist.tile([128, MT], f32, tag="base")

            sumsq_all = persist.tile([128, MT], f32, tag="sumsq")
            s1_all = persist.tile([128, MT], f32, tag="s1")
            nrm_all = persist.tile([128, MT], f32, tag="nrm")
            rn_all = persist.tile([128, MT], f32, tag="rn")
            ms_all = persist.tile([128, MT], f32, tag="ms")
            sqd = persist.tile([128, HW], bf16, tag="sqd")
            xw_pool = ctx.enter_context(tc.tile_pool(name="xw", bufs=4))
            dn_pool = ctx.enter_context(tc.tile_pool(name="dn", bufs=2))
            stats = ctx.enter_context(tc.tile_pool(name="stats", bufs=6))
            xs_pool = ctx.enter_context(tc.tile_pool(name="xs", bufs=2))
            row_pool = ctx.enter_context(tc.tile_pool(name="rows", bufs=2))
            scr_pool = ctx.enter_context(tc.tile_pool(name="scr", bufs=2))

            ps_pool = ctx.enter_context(
                tc.tile_pool(name="ps", bufs=2, space="PSUM")
            )
            psg_pool = ctx.enter_context(
                tc.tile_pool(name="psg", bufs=2, space="PSUM")
            )
            pm_pool = ctx.enter_context(
                tc.tile_pool(name="pm", bufs=1, space="PSUM")
            )
            pt_pool = ctx.enter_context(
                tc.tile_pool(name="pt", bufs=1, space="PSUM")
            )

            if repeat > 1:
                ctx.enter_context(tc.For_i(0, repeat, 1))

            def stage0_load(m):
                xw = xw_pool.tile([128, HW], bf16, tag="xw")
                nc.sync.dma_start(out=xw, in_=x1s[m * 128:(m + 1) * 128, :])
                nc.scalar.activation(
                    out=sqd, in_=xw, func=AF.Square,
                    accum_out=sumsq_all[:, m:m + 1],
                )

            def stage0_sum(m):
                xw = xw_pool.tile([128, HW], bf16, tag="xw")
                nc.sync.dma_start(out=xw, in_=x1s[m * 128:(m + 1) * 128, :])
                nc.scalar.activation(
                    out=sqd, in_=xw, func=AF.Copy,
                    accum_out=s1_all[:, m:m + 1],
                )

            def stage0_stats():
                # one batched op per stat over all 32 m-tiles
                nc.scalar.activation(out=nrm_all, in_=sumsq_all, func=AF.Sqrt)
                nc.vector.reciprocal(out=rn_all, in_=nrm_all)
                nc.vector.tensor_scalar(
                    out=ms_all, in0=s1_all, scalar1=1.0 / HW, scalar2=None,
                    op0=OP.mult,
                )
                mq_all = stats.tile([128, MT], f32, tag="mq")
                nc.vector.tensor_tensor(
                    out=mq_all, in0=ms_all, in1=rn_all, op=OP.mult
                )
                msq_all = stats.tile([128, MT], f32, tag="msq")
                nc.vector.tensor_tensor(
                    out=msq_all, in0=mq_all, in1=mq_all, op=OP.mult
                )
                nc.vector.tensor_scalar(
                    out=base_t, in0=msq_all, scalar1=-float(HW), scalar2=1.0,
                    op0=OP.mult, op1=OP.add,
                )

            def stage0_m(m):
                xw = xw_pool.tile([128, HW], bf16, tag="xw")
                nc.sync.dma_start(out=xw, in_=x1s[m * 128:(m + 1) * 128, :])
                dn = dn_pool.tile([128, HW], f32r, tag="dn")
                nc.vector.tensor_scalar(
                    out=dn, in0=xw, scalar1=ms_all[:, m:m + 1],
                    scalar2=rn_all[:, m:m + 1],
                    op0=OP.subtract, op1=OP.mult,
                )
                nc.vector.tensor_copy(out=d_res[:, m, 0, :], in_=dn[:, 0:392])
                nc.vector.tensor_copy(out=d_res[:, m, 1, :], in_=dn[:, 392:784])
                for kt, (koff, klen) in enumerate(KT):
                    pt = pt_pool.tile([128, 128], f32r, tag="pt")
                    nc.tensor.transpose(
                        out=pt[:klen, :], in_=dn[:, koff:koff + klen],
                        identity=ident,
                    )
                    dst = dtT[:klen, m, kt, :]
                    if kt % 2 == 0:
                        nc.scalar.mul(out=dst, in_=pt[:klen, :], mul=SD)
                    else:
                        nc.vector.tensor_scalar(
                            out=dst, in0=pt[:klen, :], scalar1=SD,
                            scalar2=None, op0=OP.mult,
                        )

            def gram_class(n):
                xs = xs_pool.tile([128, RTN, HW], fp8, tag="xs")
                for rt in range(RTN):
                    nc.sync.dma_start(out=xs[:, rt, :], in_=x2d[n, rt, :, :])
                srow = row_pool.tile([1, HW], fp8, tag="srow")
                ssrow = row_pool.tile([1, HW], fp8, tag="ssrow")
                for qi, (qoff, qlen) in enumerate(QT):
                    pm = pm_pool.tile([1, 512], f32, tag="pm")
                    for r in range(RTN // 2):
                        nc.tensor.matmul(
                            pm[:1, :qlen],
                            lhsT=ones2[:, :, 0:1],
                            rhs=xs[:, 2 * r:2 * r + 2, qoff:qoff + qlen],
                            start=(r == 0), stop=(r == RTN // 2 - 1),
                            perf_mode=DRM,
                        )
                    qs = slice(qoff, qoff + qlen)
                    nc.scalar.mul(out=srow[:, qs], in_=pm[:1, :qlen], mul=1.0)
                    nc.scalar.mul(
                        out=ssrow[:, qs], in_=pm[:1, :qlen], mul=-1.0 / NR
                    )
                for mc, (mcoff, mclen) in enumerate(KT):
                    for qi, (qoff, qlen) in enumerate(QT):
                        psg = psg_pool.tile([128, 512], f32, tag="psg")
                        for r in range(RTN // 2):
                            nc.tensor.matmul(
                                psg[:mclen, :qlen],
                                lhsT=xs[:, 2 * r:2 * r + 2,
                                        mcoff:mcoff + mclen],
                                rhs=xs[:, 2 * r:2 * r + 2, qoff:qoff + qlen],
                                start=(r == 0), stop=False,
                                perf_mode=DRM,
                            )
                        has_diag = (mcoff < qoff + qlen
                                    and qoff < mcoff + mclen)
                        nc.tensor.matmul(
                            psg[:mclen, :qlen],
                            lhsT=ssrow[:1, mcoff:mcoff + mclen],
                            rhs=srow[:1, qoff:qoff + qlen],
                            start=False, stop=not has_diag,
                            skip_group_check=True,
                        )
                        if has_diag:
                            s_off = EYE_OFF - mcoff + qoff
                            nc.tensor.matmul(
                                psg[:mclen, :qlen],
                                lhsT=ai[:, :mclen],
                                rhs=eye[:, s_off:s_off + qlen],
                                start=False, stop=True,
                                skip_group_check=True,
                            )
                        dst = cov[:mclen, n, mc, qoff:qoff + qlen]
                        t_off = TRI_OFF - mcoff + qoff
                        nc.vector.scalar_tensor_tensor(
                            out=dst, in0=psg[:mclen, :qlen],
                            scalar=SC / (NR - 1),
                            in1=tri[:mclen, t_off:t_off + qlen],
                            op0=OP.mult, op1=OP.mult,
                        )

            # ---- Phase A: batched stage-0 with gram classes interleaved ----
            for i in range(MT):
                stage0_load(i)
                if i >= MT - 2 - NW and i < MT - 2:
                    gram_class(i - (MT - 2 - NW))
            for i in range(MT):
                stage0_sum(i)
            stage0_stats()
            for i in range(MT):
                stage0_m(i)

            # ---- Phase B: sim = (D @ V) . D row-reduced ----
            for n in range(NW if abl != "nophaseb" else 0):
                for m in range(MT):
                    ps = ps_pool.tile([128, 2, 512], f32, tag="ps")
                    for qi, (qoff, qlen) in enumerate(QT):
                        if abl == "nomm":
                            break
                        for t in range(NKT8 // 2):
                            nc.tensor.matmul(
                                ps[:, qi, :qlen],
                                lhsT=dtT[:, m, 2 * t:2 * t + 2, :],
                                rhs=cov[:, n, 2 * t:2 * t + 2,
                                        qoff:qoff + qlen],
                                start=(t == 0), stop=(t == NKT8 // 2 - 1),
                                perf_mode=DRM,
                            )
                    if abl == "nostt":
                        continue
                    scr = scr_pool.tile([128, 2, 392], bf16, tag="scr")
                    nc.vector.scalar_tensor_tensor(
                        out=scr,
                        in0=ps[:, :, :392],
                        scalar=1.0 / (SD * SC),
                        in1=d_res[:, m, :, :],
                        op0=OP.mult, op1=OP.mult,
                        accum_out=out_acc[:, m, n:n + 1],
                    )

            for m in range(MT):
                nc.vector.tensor_scalar(
                    out=out_acc[:, m, :], in0=out_acc[:, m, :],
                    scalar1=base_t[:, m:m + 1], scalar2=None, op0=OP.add,
                )
                nc.sync.dma_start(out=out[m], in_=out_acc[:, m, :])
            if debug:
                nc.sync.dma_start(out=dbg_dtT[:, :, :, :], in_=dtT)
                nc.sync.dma_start(out=dbg_cov[:, :, :, :], in_=cov)
                nc.sync.dma_start(out=dbg_base[:, :], in_=base_t)
                nc.sync.dma_start(out=dbg_dres[:, :, :, :], in_=d_res)

    nc.finalize()
    return nc


def get_program():
    key = "nc"
    if key not in _STATE:
        _STATE[key] = _build_program()
    return _STATE[key]


def make_in_maps(x1, x2):
    import ml_dtypes

    x1f = np.asarray(x1, dtype=np.float32).reshape(B * C, HW)
    x1b = np.ascontiguousarray(x1f).astype(ml_dtypes.bfloat16)
    x2f = np.asarray(x2, dtype=np.float32).reshape(NW, RTN, 128, HW)
    x2q = np.ascontiguousarray(x2f).astype(ml_dtypes.float8_e4m3)
    return [
        {"x1s": x1b[c * NI:(c + 1) * NI], "x2": x2q}
        for c in range(N_CORES)
    ]


def assemble_output(core_outs):
    # per-core (MT, 128, NW) -> (BSH, NW*C); concat over cores -> (B, NW*C)
    parts = [
        o.reshape(NI, NW).reshape(BSH, C, NW).transpose(0, 2, 1)
        .reshape(BSH, NW * C)
        for o in core_outs
    ]
    return np.ascontiguousarray(np.concatenate(parts, axis=0), dtype=np.float32)


def kernel(x1, x2):
    from concourse.bass_utils import run_bass_kernel_spmd

    nc = get_program()
    in_maps = make_in_maps(x1, x2)
    res = run_bass_kernel_spmd(nc, in_maps, list(range(N_CORES)))
    return assemble_output([res.results[i]["out"] for i in range(N_CORES)])
